# revision 1
# baseline (speedup 1.0000x reference)
"""CondInst dynamic mask head on 8 Trainium2 NeuronCores.

Math (per instance i with gathered params):
    x_i   = [rel_i (2,HW); feats_b (8,HW)]
    h1    = relu(w0_i @ x_i + b0_i)        # (8,HW)
    h2    = relu(w1_i @ h1 + b1_i)         # (8,HW)
    out_i = sigmoid(w2_i @ h2 + b2_i)      # (1,HW)

rel_i = (loc_i - coords)/128 is affine in the shared coords map, so it is
folded into a shared X = [coords/128; feats; ones] with per-instance
effective weights Ahat_i = [-w0r_i | w0f_i] and bias c0_i = b0_i + w0r_i@loc_i/128
(the bias rides the ones-row of X).

Sharding: core c -> batch b=c//2, L-half c%2 (8192 cols), all 100 instances.
Instances are grouped in 25 blocks of 4 (32 rows of 4x8 channels) mapped onto
32x32 PE-array tiles; matmuls run in bfloat16.

L2 uses per-block narrow [4,256] psum outputs at partition 4t+j == instance
index (no accumulation chains): one sigmoid op per chunk pair covers all 100
instances, and the output DMA is 4 large [100, 2048] transfers from a
persistent SBUF buffer.
"""

import os
import sys

import numpy as np

sys.path.insert(0, "/opt/trn_rl_repo")
os.environ.setdefault("MYCRO_LOCAL_CACHE", "1")

B, K, C, H, Wd = 4, 100, 8, 128, 128
HW = H * Wd
P_ = (C + 2) * C + C + C * C + C + C + 1  # 169
LC = HW // 2          # 8192 L-columns per core
WCH = 256             # L-chunk (free dim) per matmul / psum round
NCHUNK = LC // WCH    # 32
NB = 25               # instance blocks of 4
NCORE = 8

MM_DTYPE = os.environ.get("CONDINST_MM_DTYPE", "bfloat16")

_PROGRAM = None  # cached (nc, meta)


# ---------------------------------------------------------------- mappings
def l0_map(t):
    """block t -> (row_group r, col_group c, slot s, stationary col-block k)."""
    if t < 16:
        r, c, s = t // 4, t % 4, 0
    else:
        u = t - 16
        r, c, s = u // 4, u % 4, 1
    return r, c, s, c + 4 * s


def l1_map(t):
    """block t -> (psum bank-group q=row_group, col_group c1, slot s, col-block k1).

    c1 = t//8 so that all 8 blocks of an L2 output col group share one h2
    partition group -> the L2 accumulation chain stays on a single PE tile
    position (32*c1, 32*c1), like a classic diagonal chain.
    """
    if t < 16:
        q, k1 = t % 4, t // 4
    else:
        u = t - 16
        q, k1 = u % 4, 4 + u // 4
    return q, t // 8, (t % 8) // 4, k1


def l2_map(t):
    """block t -> (w2s row base, w2s col base) for its [32,32] chain slab.

    L2 output for block t lands at psum partitions [4t, 4t+4): partition ==
    instance index.  Each col group g = t//8 is written by an 8-MM
    accumulation chain at position (32*c1, 32*g); slab col 4*(t%8)+j drives
    psum partition 4t+j.
    """
    _, c1, _, _ = l1_map(t)
    return 32 * c1, 32 * (t % 8)


# ---------------------------------------------------------------- host prep
def _prep_inputs(seg_feat, conv_weight, ind):
    seg_feat = np.asarray(seg_feat, dtype=np.float32)
    conv_weight = np.asarray(conv_weight, dtype=np.float32)
    ind_np = np.asarray(ind)
    ind64 = ind_np.astype(np.int64)

    cw = conv_weight.reshape(B, P_, HW)
    # params[b, k, p] = cw[b, p, ind[b, k]]
    params = np.take_along_axis(cw, ind64[:, None, :], axis=2)  # [B, P, K]
    params = params.transpose(0, 2, 1)  # [B, K, P]

    w0 = params[..., 0:80].reshape(B, K, C, C + 2)
    w1 = params[..., 80:144].reshape(B, K, C, C)
    w2 = params[..., 144:152].reshape(B, K, 1, C)
    b0 = params[..., 152:160]
    b1 = params[..., 160:168]
    b2 = params[..., 168:169]

    xi = (ind64 % Wd).astype(np.float32)
    yi = (ind64 // Wd).astype(np.float32)
    loc = np.stack([xi, yi], axis=-1)  # [B, K, 2]

    w0r = w0[..., 0:2]   # [B, K, 8, 2]
    w0f = w0[..., 2:10]  # [B, K, 8, 8]
    ahat = np.concatenate([-w0r, w0f], axis=-1)  # [B, K, 8, 10]
    c0 = b0 + np.einsum("bkoc,bkc->bko", w0r, loc) / 128.0  # [B, K, 8]

    lin = np.arange(HW, dtype=np.float32)
    coords_x = (lin % Wd) / 128.0
    coords_y = np.floor(lin / Wd) / 128.0

    in_maps = []
    for core in range(NCORE):
        b = core // 2
        lo = (core % 2) * LC
        sl = slice(lo, lo + LC)

        xrep = np.empty((11, LC), dtype=np.float32)
        xrep[0] = coords_x[sl]
        xrep[1] = coords_y[sl]
        xrep[2:10] = seg_feat[b].reshape(C, HW)[:, sl]
        xrep[10] = 1.0

        w0s = np.zeros((128, 256), dtype=np.float32)
        w1s = np.zeros((128, 256), dtype=np.float32)
        w2s = np.zeros((128, 256), dtype=np.float32)
        b1sb = np.zeros((128, 8), dtype=np.float32)
        b2sb = np.zeros((128, 1), dtype=np.float32)

        for t in range(NB):
            r0, _, _, k0 = l0_map(t)
            q1, c1, s1, k1 = l1_map(t)
            rb2, cb2 = l2_map(t)
            for j in range(4):
                inst = 4 * t + j
                # L0 stationary: [11, 32] at rows 32r0, cols 32k0 (+8j per inst)
                w0s[32 * r0:32 * r0 + 10, 32 * k0 + 8 * j:32 * k0 + 8 * j + 8] = \
                    ahat[b, inst].T
                w0s[32 * r0 + 10, 32 * k0 + 8 * j:32 * k0 + 8 * j + 8] = c0[b, inst]
                # L1 stationary: blockdiag W1^T at rows 32q1.  The stationary
                # col within the slab must equal the psum partition offset
                # within the col group: 8*j stays correct since block t owns
                # partitions 32c1+8j+ch via slot j.
                w1s[32 * q1 + 8 * j:32 * q1 + 8 * j + 8,
                    32 * k1 + 8 * j:32 * k1 + 8 * j + 8] = w1[b, inst].T
                # L1 bias vector for psum partition 32c1 + 8j + ch, column 2q1+s1
                b1sb[32 * c1 + 8 * j:32 * c1 + 8 * j + 8, 2 * q1 + s1] = b1[b, inst]
                # L2 stationary: block t occupies rows rb2..rb2+32 of its
                # chain's [64,32] slab; col 4*(t%8)+j drives psum part. 4t+j
                w2s[rb2 + 8 * j:rb2 + 8 * j + 8,
                    cb2 + 4 * (t % 8) + j] = w2[b, inst, 0]
                # sigmoid bias at psum partition 4t + j == instance index
                b2sb[4 * t + j, 0] = b2[b, inst, 0]

        if MM_DTYPE == "bfloat16":
            import ml_dtypes
            bf16 = ml_dtypes.bfloat16
            xrep = xrep.astype(bf16)
            w0s = w0s.astype(bf16)
            w1s = w1s.astype(bf16)
            w2s = w2s.astype(bf16)
        in_maps.append({
            "xrep": xrep, "w0s": w0s, "w1s": w1s, "w2s": w2s,
            "b1sb": b1sb, "b2sb": b2sb,
        })
    return in_maps, ind_np.dtype


# ---------------------------------------------------------------- program
def build_program():
    global _PROGRAM
    if _PROGRAM is not None:
        return _PROGRAM

    import concourse.tile as tile
    from concourse import bacc, mybir

    nc = bacc.Bacc("TRN2", target_bir_lowering=False, debug=False)
    f32 = mybir.dt.float32
    mm_dt = getattr(mybir.dt, MM_DTYPE)
    Relu = mybir.ActivationFunctionType.Relu
    Sigmoid = mybir.ActivationFunctionType.Sigmoid
    Alu = mybir.AluOpType

    xrep_h = nc.dram_tensor("xrep", [11, LC], mm_dt, kind="ExternalInput")
    w0s_h = nc.dram_tensor("w0s", [128, 256], mm_dt, kind="ExternalInput")
    w1s_h = nc.dram_tensor("w1s", [128, 256], mm_dt, kind="ExternalInput")
    w2s_h = nc.dram_tensor("w2s", [128, 256], mm_dt, kind="ExternalInput")
    b1_h = nc.dram_tensor("b1sb", [128, 8], f32, kind="ExternalInput")
    b2_h = nc.dram_tensor("b2sb", [128, 1], f32, kind="ExternalInput")
    out_h = nc.dram_tensor("out_shard", [4 * NB, LC], f32, kind="ExternalOutput")

    GRP = 512  # output DMA group width (cols)

    with tile.TileContext(nc) as tc:
        with (
            tc.tile_pool(name="const", bufs=1) as cpool,
            tc.tile_pool(name="h1p", bufs=8) as h1pool,
            tc.tile_pool(name="h2p", bufs=8) as h2pool,
            tc.tile_pool(name="ps", bufs=4, space="PSUM") as pspool,
        ):
            xrep = cpool.tile([128, LC], mm_dt, tag="xrep")
            w0s = cpool.tile([128, 256], mm_dt, tag="w0s")
            w1s = cpool.tile([128, 256], mm_dt, tag="w1s")
            w2s = cpool.tile([128, 256], mm_dt, tag="w2s")
            b1s = cpool.tile([128, 8], f32, tag="b1s")
            b2s = cpool.tile([128, 1], f32, tag="b2s")
            outb = cpool.tile([128, LC], f32, tag="outb")

            nc.gpsimd.dma_start(w0s[:], w0s_h[:])
            for r in range(4):
                nc.gpsimd.dma_start(xrep[32 * r:32 * r + 11, :], xrep_h[:])
            nc.gpsimd.dma_start(w1s[:], w1s_h[:])
            nc.gpsimd.dma_start(w2s[:], w2s_h[:])
            nc.gpsimd.dma_start(b1s[:], b1_h[:])
            nc.gpsimd.dma_start(b2s[:], b2_h[:])

            # HAM pre-warm: ~4.5us of dummy matmuls on w0s during the input-DMA
            # window flips the PE clock gate to 8/8 (2.4 GHz) before the real
            # pipeline starts; the dense main loop then keeps it warm.
            warm = pspool.tile([128, 512], f32, tag="pc", bufs=1, name="warm")
            for i in range(20):
                nc.tensor.matmul(
                    warm[0:32, 0:256], w0s[0:11, 0:32], w0s[0:11, 0:256],
                    tile_position=(0, 0),
                )

            # Software-pipelined emission: in iteration `it` the PE stream is
            # [L0 MMs of chunk it][L1 MMs of chunk it-1][L2 MMs of chunk it-2]
            # so every matmul's inputs were evacuated a full stage earlier and
            # PE / ScalarE / VectorE all run without cross-stage stalls.
            h1_by, h2_by, p2_by = {}, {}, {}

            for it in range(NCHUNK + 3):
                a, b2c, c2c = it, it - 1, it - 2

                # ---- L0 matmuls for chunk a
                if a < NCHUNK:
                    fl = slice(a * WCH, (a + 1) * WCH)
                    p0 = [pspool.tile([128, 512], f32, tag="pp", bufs=7,
                                      name=f"p0_{a}_{r}") for r in range(4)]
                    for t in range(NB):
                        r0, c0_, s0, k0 = l0_map(t)
                        nc.tensor.matmul(
                            p0[r0][32 * c0_:32 * c0_ + 32,
                                   256 * s0:256 * s0 + 256],
                            (w0s[32 * r0:32 * r0 + 11, 32 * k0:32 * k0 + 32]),
                            (xrep[32 * r0:32 * r0 + 11, fl]),
                            tile_position=(32 * r0, 32 * c0_),
                        )

                # ---- sigmoid + group DMA for the pair ending at chunk it-3
                d = it - 3
                if 0 <= d < NCHUNK and d % 2 == 1:
                    p2 = p2_by.pop(d - 1)
                    fl2 = slice((d - 1) * WCH, (d + 1) * WCH)
                    nc.scalar.activation(outb[0:100, fl2], p2[0:100, :],
                                         Sigmoid, bias=b2s[0:100, :])
                    # every 4 pairs, ship a [100, 2048] slab to HBM
                    if (d + 1) % (GRP // WCH) == 0:
                        g0 = (d + 1) * WCH - GRP
                        nc.gpsimd.dma_start(out_h[0:100, g0:g0 + GRP],
                                            outb[0:100, g0:g0 + GRP])

                # ---- L1 matmuls for chunk b2c
                if 0 <= b2c < NCHUNK:
                    h1 = h1_by[b2c]
                    p1 = [pspool.tile([128, 512], f32, tag="pp", bufs=7,
                                      name=f"p1_{b2c}_{q}") for q in range(4)]
                    for t in range(NB):
                        r0, c0_, s0, _ = l0_map(t)
                        q1, c1, s1, k1 = l1_map(t)
                        nc.tensor.matmul(
                            p1[q1][32 * c1:32 * c1 + 32,
                                   256 * s1:256 * s1 + 256],
                            (w1s[32 * q1:32 * q1 + 32, 32 * k1:32 * k1 + 32]),
                            (h1[r0][32 * c0_:32 * c0_ + 32,
                                    256 * s0:256 * s0 + 256]),
                            tile_position=(32 * q1, 32 * c1),
                        )

                # ---- h1 = relu(p0) evacuation for chunk a (4 wide ops)
                if a < NCHUNK:
                    h1 = [h1pool.tile([128, 512], mm_dt, tag=f"h1_{r}", bufs=4,
                                      name=f"h1_{a}_{r}") for r in range(4)]
                    h1_by[a] = h1
                    nc.scalar.activation(h1[0][:], p0[0][:], Relu)
                    nc.vector.tensor_scalar_max(h1[1][:], p0[1][:], 0.0)
                    nc.scalar.activation(h1[2][:, 0:256], p0[2][:, 0:256], Relu)
                    nc.vector.tensor_scalar_max(h1[2][0:32, 256:512],
                                                p0[2][0:32, 256:512], 0.0)
                    nc.vector.tensor_scalar_max(h1[3][:, 0:256],
                                                p0[3][:, 0:256], 0.0)

                # ---- L2 matmuls for chunk c2c: 4 concurrent 8-deep chains,
                # one per psum col group; partition 4t+j == instance index
                if 0 <= c2c < NCHUNK:
                    pair = c2c % 2
                    if pair == 0:
                        p2_by[c2c] = pspool.tile([128, 512], f32, tag="pc",
                                                 bufs=1, name=f"p2_{c2c}")
                    p2 = p2_by[c2c - pair]
                    h2 = h2_by.pop(c2c)
                    for w in range(8):
                        for g in range(4):
                            t = 8 * g + w
                            if t >= NB:
                                continue
                            q1, c1, s1, _ = l1_map(t)
                            rb2, cb2 = l2_map(t)
                            nc.tensor.matmul(
                                p2[32 * g:32 * g + 32,
                                   256 * pair:256 * pair + 256],
                                (w2s[rb2:rb2 + 32, cb2:cb2 + 32]),
                                (h2[q1][32 * c1:32 * c1 + 32,
                                        256 * s1:256 * s1 + 256]),
                                start=(w == 0),
                                stop=(w == 7 or t == NB - 1),
                                skip_group_check=True,
                                tile_position=(32 * c1, 32 * g),
                            )

                # ---- h2 = relu(p1 + b1) evacuation for chunk b2c
                if 0 <= b2c < NCHUNK:
                    h2 = [h2pool.tile([128, 512], mm_dt, tag=f"h2_{q}", bufs=4,
                                      name=f"h2_{b2c}_{q}") for q in range(4)]
                    h2_by[b2c] = h2
                    s0parts = [128, 96, 96, 96]
                    s1parts = [96, 96, 96, 96]
                    for q in range(4):
                        np0 = s0parts[q]
                        bias0 = b1s[0:np0, 2 * q:2 * q + 1]
                        if q < 3:
                            nc.scalar.activation(h2[q][0:np0, 0:256],
                                                 p1[q][0:np0, 0:256], Relu,
                                                 bias=bias0)
                        else:
                            nc.vector.tensor_scalar(
                                h2[q][0:np0, 0:256], p1[q][0:np0, 0:256],
                                bias0, 0.0, Alu.add, Alu.max)
                        np1 = s1parts[q]
                        bias1 = b1s[0:np1, 2 * q + 1:2 * q + 2]
                        if q == 0:
                            nc.scalar.activation(h2[q][0:np1, 256:512],
                                                 p1[q][0:np1, 256:512], Relu,
                                                 bias=bias1)
                        else:
                            nc.vector.tensor_scalar(
                                h2[q][0:np1, 256:512], p1[q][0:np1, 256:512],
                                bias1, 0.0, Alu.add, Alu.max)

    nc.compile()
    _PROGRAM = nc
    return nc


# ---------------------------------------------------------------- entry
def kernel(seg_feat, conv_weight, ind):
    from concourse.bass_utils import run_bass_kernel_spmd

    in_maps, ind_dtype = _prep_inputs(seg_feat, conv_weight, ind)
    nc = build_program()
    res = run_bass_kernel_spmd(nc, in_maps, list(range(NCORE)))
    out = np.empty((B, K, HW), dtype=np.float32)
    for core in range(NCORE):
        b = core // 2
        lo = (core % 2) * LC
        out[b, :, lo:lo + LC] = res.results[core]["out_shard"]
    return out.reshape(B, K, H, Wd)

